# revision 9
# baseline (speedup 1.0000x reference)
"""Cross-attention Trainium2 kernel (nn_CrossAttention).

Reference computation (per batch b):
    q = Wq @ x1 + bq            [32, N]     (N = 64*64 = 4096)
    k = Wk @ x2 + bk            [32, N]
    v = Wv @ x2 + bv            [256, N]
    attn = softmax(q^T k, axis over keys m)     [N, N]
    out[c, n] = sum_m v[c, m] attn[n, m]        [256, N]

Sharding: 8 cores = 4 batches x 2 query-halves (2048 queries per core, all
4096 keys).  Each core runs the same NEFF on its own input slice; softmax
rows are complete within a core so no cross-core communication is needed.

Per-core kernel layout choices:
  * S^T tiles [keys m on partitions, queries n on free dim] so that the
    second matmul (attn @ V) can consume exp(S^T) directly from SBUF with m
    as the contraction dim -- no transposes anywhere.
  * Q and K are produced replicated 4x across partition groups (Wq/Wk
    stacked 4x on the host) so the D=32-contraction QK^T matmuls can be
    row-packed 4-per-PE-array via tile_position.
  * Row sums of exp(S^T) (softmax denominators) are computed on the PE with
    col-packed ones-matmuls (4 concurrent M=32 tiles), then combined +
    broadcast to all partitions with a single (1/32)-scaled ones matmul.
  * All matmuls use float32r (full fp32 data, reduced-precision PE mode,
    1 cycle/row for free dim >= 256 vs 4 cycles/row for plain fp32).
  * Softmax skips the max-subtraction: logits are ~N(0, 32), |s| < ~45
    for this problem size, exp() stays comfortably inside fp32 range.
  * bv is folded in at the end: out += bv (softmax rows sum to 1).
"""

import sys

for _p in (
    "/root/.axon_site",
    "/root/.axon_site/_ro/trn_rl_repo",
    "/root/.axon_site/_ro/pypackages",
):
    if _p not in sys.path:
        sys.path.append(_p)

import numpy as np

import concourse.bass as bass
from concourse import bacc
import concourse.tile as tile
from concourse import mybir
from concourse import bass_utils

B = 4
C = 256          # value/input channels
D = 32           # q/k channels
N = 4096         # keys per batch (64*64)
NQ = 2048        # queries per core (half a batch)
NT = 512         # query tile (free dim of S^T / output matmuls)
NNT = NQ // NT   # 4 query tiles
NSC = 8          # key super-chunks of 512 (4 x 128) keys
F32 = mybir.dt.float32
F32R = mybir.dt.float32r
AFT = mybir.ActivationFunctionType


def attn_tile_kernel(tc, out, x1, x2, wq4t, wk4t, wvt, bq4, bk4, bv, ones_c):
    nc = tc.nc

    with (
        tc.tile_pool(name="consts", bufs=1) as consts,
        tc.tile_pool(name="bigbuf", bufs=1) as bigbuf,
        tc.tile_pool(name="ptbuf", bufs=2) as ptbuf,
        tc.tile_pool(name="finbuf", bufs=2) as finbuf,
    ):
        # ---- constants / weights -------------------------------------
        ones_rs = consts.tile([128, 128], F32R, name="ones_rs")
        nc.sync.dma_start(out=ones_rs, in_=ones_c)

        bq4_sb = consts.tile([128, 1], F32, name="bq4_sb")
        nc.sync.dma_start(out=bq4_sb, in_=bq4)
        bk4_sb = consts.tile([128, 1], F32, name="bk4_sb")
        nc.sync.dma_start(out=bk4_sb, in_=bk4)
        bv_sb = []
        for cc in range(2):
            t = consts.tile([128, 1], F32, name=f"bv_sb{cc}")
            nc.sync.dma_start(out=t, in_=bv[cc * 128 : (cc + 1) * 128, :])
            bv_sb.append(t)

        wq4t_sb, wk4t_sb, wvt_sb = [], [], []
        for kc in range(2):
            rows = slice(kc * 128, (kc + 1) * 128)
            t = consts.tile([128, 128], F32R, name=f"wq4t_sb{kc}")
            nc.sync.dma_start(out=t, in_=wq4t[rows, :])
            wq4t_sb.append(t)
            t = consts.tile([128, 128], F32R, name=f"wk4t_sb{kc}")
            nc.sync.dma_start(out=t, in_=wk4t[rows, :])
            wk4t_sb.append(t)
            t = consts.tile([128, 256], F32R, name=f"wvt_sb{kc}")
            nc.sync.dma_start(out=t, in_=wvt[rows, :])
            wvt_sb.append(t)

        # ---- feature maps --------------------------------------------
        x1_sb, x2_sb = [], []
        for kc in range(2):
            rows = slice(kc * 128, (kc + 1) * 128)
            t = bigbuf.tile([128, NQ], F32R, name=f"x1_sb{kc}")
            for blk in range(2):
                cols = slice(blk * 1024, (blk + 1) * 1024)
                nc.sync.dma_start(out=t[:, cols], in_=x1[rows, cols])
            x1_sb.append(t)
            t = bigbuf.tile([128, N], F32R, name=f"x2_sb{kc}")
            for blk in range(4):
                cols = slice(blk * 1024, (blk + 1) * 1024)
                nc.sync.dma_start(out=t[:, cols], in_=x2[rows, cols])
            x2_sb.append(t)

        q4_sb = bigbuf.tile([128, NQ], F32R, name="q4_sb")
        k4_sb = bigbuf.tile([128, N], F32R, name="k4_sb")
        vt_sb = bigbuf.tile([128, C * N // 128], F32R, name="vt_sb")  # [128, 8192]

        # ---- prep: projections ---------------------------------------
        with tc.tile_pool(name="prep_psum", bufs=2, space="PSUM") as pp:
            # Q4 [128, 2048] = (Wq stacked 4x) @ x1, then +bq
            psum_q = pp.tile([128, NQ], F32, name="psum_q", tag="prep")
            for t4 in range(NNT):
                cols = slice(t4 * NT, (t4 + 1) * NT)
                for kc in range(2):
                    nc.tensor.matmul(
                        psum_q[:, cols],
                        lhsT=(wq4t_sb[kc]),
                        rhs=(x1_sb[kc][:, cols]),
                        start=(kc == 0),
                        stop=(kc == 1),
                    )
            nc.vector.tensor_scalar_add(q4_sb, psum_q, bq4_sb)

            # K4 [128, 4096] in two passes
            for p in range(2):
                psum_k = pp.tile([128, NQ], F32, name=f"psum_k{p}", tag="prep")
                for t4 in range(NNT):
                    cols = slice(t4 * NT, (t4 + 1) * NT)
                    src = slice(p * NQ + t4 * NT, p * NQ + (t4 + 1) * NT)
                    for kc in range(2):
                        nc.tensor.matmul(
                            psum_k[:, cols],
                            lhsT=(wk4t_sb[kc]),
                            rhs=(x2_sb[kc][:, src]),
                            start=(kc == 0),
                            stop=(kc == 1),
                        )
                nc.vector.tensor_scalar_add(
                    k4_sb[:, p * NQ : (p + 1) * NQ], psum_k, bk4_sb
                )

            # V^T: vt_sb[:, mc*256 + c] = V[c, mc*128 + p] ; 4 passes of 8
            # m-chunks.  (bv is added at the very end of the kernel.)
            for p in range(4):
                psum_v = pp.tile([128, NQ], F32, name=f"psum_v{p}", tag="prep")
                for m8 in range(8):
                    mc = 8 * p + m8
                    for kc in range(2):
                        nc.tensor.matmul(
                            psum_v[:, m8 * 256 : (m8 + 1) * 256],
                            lhsT=(x2_sb[kc][:, mc * 128 : (mc + 1) * 128]),
                            rhs=(wvt_sb[kc]),
                            start=(kc == 0),
                            stop=(kc == 1),
                        )
                dst = vt_sb[:, p * 2048 : (p + 1) * 2048]
                if p % 2 == 0:
                    nc.scalar.copy(dst, psum_v)
                else:
                    nc.vector.tensor_copy(dst, psum_v)

        # ---- main attention loop -------------------------------------
        with (
            tc.tile_pool(name="s_psum", bufs=1, space="PSUM") as sp,
            tc.tile_pool(name="o_psum", bufs=1, space="PSUM") as op,
        ):
            for nt in range(NNT):
                qcols = slice(nt * NT, (nt + 1) * NT)
                psum_o0 = op.tile([128, NT], F32, name=f"po0_{nt}", tag="o0")
                psum_o1 = op.tile([128, NT], F32, name=f"po1_{nt}", tag="o1")
                psum_rs = op.tile([128, NT], F32, name=f"prs_{nt}", tag="rs")
                psum_o = [psum_o0, psum_o1]

                def _emit_av(sc, pt):
                    first, last = sc == 0, sc == NSC - 1
                    for j in range(4):
                        pcols = slice(j * NT, (j + 1) * NT)
                        nc.tensor.matmul(
                            psum_rs,
                            lhsT=(ones_rs),
                            rhs=(pt[:, pcols]),
                            start=(first and j == 0),
                            stop=(last and j == 3),
                        )
                    for j in range(4):
                        mc = 4 * sc + j
                        pcols = slice(j * NT, (j + 1) * NT)
                        for cc in range(2):
                            nc.tensor.matmul(
                                psum_o[cc],
                                lhsT=(
                                    vt_sb[:, mc * 256 + cc * 128 : mc * 256 + (cc + 1) * 128]
                                ),
                                rhs=(pt[:, pcols]),
                                start=(first and j == 0),
                                stop=(last and j == 3),
                            )

                prev = None
                for sc in range(NSC):
                    # S^T super-chunk: 4 row-packed matmuls, contract dim 32
                    psum_s = sp.tile([128, 4 * NT], F32, name=f"ps_{nt}_{sc}", tag="s")
                    for j in range(4):
                        mc = 4 * sc + j
                        rowg = slice(32 * j, 32 * (j + 1))
                        nc.tensor.matmul(
                            psum_s[:, j * NT : (j + 1) * NT],
                            lhsT=(k4_sb[rowg, mc * 128 : (mc + 1) * 128]),
                            rhs=(q4_sb[rowg, qcols]),
                            start=True,
                            stop=True,
                            tile_position=(32 * j, 0),
                        )
                    pt = ptbuf.tile([128, 4 * NT], F32R, name=f"pt_{nt}_{sc}", tag="pt")
                    nc.scalar.activation(out=pt, in_=psum_s, func=AFT.Exp)
                    if prev is not None:
                        _emit_av(*prev)
                    prev = (sc, pt)
                _emit_av(*prev)

                # ---- finalize: normalize + bias + store ----------------
                rbc = finbuf.tile([128, NT], F32, name=f"rbc_{nt}", tag="rbc")
                nc.vector.reciprocal(rbc, psum_rs)
                for cc in range(2):
                    t_sb = finbuf.tile([128, NT], F32, name=f"t_{nt}_{cc}", tag=f"t{cc}")
                    nc.vector.tensor_mul(t_sb, psum_o[cc], rbc)
                    o_sb = finbuf.tile([128, NT], F32, name=f"o_{nt}_{cc}", tag=f"o{cc}")
                    nc.scalar.add(o_sb, t_sb, bv_sb[cc])
                    nc.sync.dma_start(
                        out=out[cc * 128 : (cc + 1) * 128, qcols], in_=o_sb
                    )


def build_nc():
    nc = bacc.Bacc("TRN2", target_bir_lowering=False, debug=False)
    x1 = nc.dram_tensor("x1", [C, NQ], F32R, kind="ExternalInput").ap()
    x2 = nc.dram_tensor("x2", [C, N], F32R, kind="ExternalInput").ap()
    wq4t = nc.dram_tensor("wq4t", [C, 128], F32R, kind="ExternalInput").ap()
    wk4t = nc.dram_tensor("wk4t", [C, 128], F32R, kind="ExternalInput").ap()
    wvt = nc.dram_tensor("wvt", [C, C], F32R, kind="ExternalInput").ap()
    bq4 = nc.dram_tensor("bq4", [128, 1], F32, kind="ExternalInput").ap()
    bk4 = nc.dram_tensor("bk4", [128, 1], F32, kind="ExternalInput").ap()
    bv = nc.dram_tensor("bv", [C, 1], F32, kind="ExternalInput").ap()
    ones_c = nc.dram_tensor("ones_c", [128, 128], F32R, kind="ExternalInput").ap()
    out = nc.dram_tensor("out", [C, NQ], F32, kind="ExternalOutput").ap()
    with tile.TileContext(nc) as tc:
        attn_tile_kernel(tc, out, x1, x2, wq4t, wk4t, wvt, bq4, bk4, bv, ones_c)
    nc.compile()
    return nc


def make_in_maps(f1, f2, Wq, bq, Wk, bk, Wv, bv):
    f1 = np.asarray(f1, dtype=np.float32)
    f2 = np.asarray(f2, dtype=np.float32)
    Wq = np.asarray(Wq, dtype=np.float32)
    Wk = np.asarray(Wk, dtype=np.float32)
    Wv = np.asarray(Wv, dtype=np.float32)
    bq = np.asarray(bq, dtype=np.float32)
    bk = np.asarray(bk, dtype=np.float32)
    bv = np.asarray(bv, dtype=np.float32)

    x1 = f1.reshape(B, C, N)
    x2 = f2.reshape(B, C, N)
    wq4t = np.ascontiguousarray(np.concatenate([Wq.T] * 4, axis=1))  # [256, 128]
    wk4t = np.ascontiguousarray(np.concatenate([Wk.T] * 4, axis=1))
    wvt = np.ascontiguousarray(Wv.T)                                 # [256, 256]
    bq4 = np.ascontiguousarray(np.tile(bq, 4).reshape(128, 1))
    bk4 = np.ascontiguousarray(np.tile(bk, 4).reshape(128, 1))
    bvv = np.ascontiguousarray(bv.reshape(C, 1))
    ones_c = np.ones((128, 128), np.float32)

    in_maps = []
    for core in range(8):
        b, h = divmod(core, 2)
        in_maps.append(
            dict(
                x1=np.ascontiguousarray(x1[b, :, h * NQ : (h + 1) * NQ]),
                x2=np.ascontiguousarray(x2[b]),
                wq4t=wq4t,
                wk4t=wk4t,
                wvt=wvt,
                bq4=bq4,
                bk4=bk4,
                bv=bvv,
                ones_c=ones_c,
            )
        )
    return in_maps


_NC_CACHE = None


def _get_nc():
    global _NC_CACHE
    if _NC_CACHE is None:
        _NC_CACHE = build_nc()
    return _NC_CACHE


def kernel(f1, f2, Wq, bq, Wk, bk, Wv, bv):
    in_maps = make_in_maps(f1, f2, Wq, bq, Wk, bk, Wv, bv)
    res = bass_utils.run_bass_kernel_spmd(_get_nc(), in_maps, core_ids=list(range(8)))
    out = np.empty((B, C, N), np.float32)
    for core in range(8):
        b, h = divmod(core, 2)
        out[b, :, h * NQ : (h + 1) * NQ] = res.results[core]["out"]
    return out.reshape(B, C, 64, 64)


# revision 10
# speedup vs baseline: 1.0375x; 1.0375x over previous
"""Cross-attention Trainium2 kernel (nn_CrossAttention).

Reference computation (per batch b):
    q = Wq @ x1 + bq            [32, N]     (N = 64*64 = 4096)
    k = Wk @ x2 + bk            [32, N]
    v = Wv @ x2 + bv            [256, N]
    attn = softmax(q^T k, axis over keys m)     [N, N]
    out[c, n] = sum_m v[c, m] attn[n, m]        [256, N]

Sharding: 8 cores = 4 batches x 2 query-halves (2048 queries per core, all
4096 keys).  Each core runs the same NEFF on its own input slice; softmax
rows are complete within a core so no cross-core communication is needed.

Per-core kernel layout choices:
  * S^T tiles [keys m on partitions, queries n on free dim] so that the
    second matmul (attn @ V) can consume exp(S^T) directly from SBUF with m
    as the contraction dim -- no transposes anywhere.
  * Q and K are produced replicated 4x across partition groups (Wq/Wk
    stacked 4x on the host) so the D=32-contraction QK^T matmuls can be
    row-packed 4-per-PE-array via tile_position.
  * Row sums of exp(S^T) (softmax denominators) are computed on the PE with
    col-packed ones-matmuls (4 concurrent M=32 tiles), then combined +
    broadcast to all partitions with a single (1/32)-scaled ones matmul.
  * All matmuls use float32r (full fp32 data, reduced-precision PE mode,
    1 cycle/row for free dim >= 256 vs 4 cycles/row for plain fp32).
  * Softmax skips the max-subtraction: logits are ~N(0, 32), |s| < ~45
    for this problem size, exp() stays comfortably inside fp32 range.
  * bv is folded in at the end: out += bv (softmax rows sum to 1).
"""

import sys

for _p in (
    "/root/.axon_site",
    "/root/.axon_site/_ro/trn_rl_repo",
    "/root/.axon_site/_ro/pypackages",
):
    if _p not in sys.path:
        sys.path.append(_p)

import numpy as np

import concourse.bass as bass
from concourse import bacc
import concourse.tile as tile
from concourse import mybir
from concourse import bass_utils

B = 4
C = 256          # value/input channels
D = 32           # q/k channels
N = 4096         # keys per batch (64*64)
NQ = 2048        # queries per core (half a batch)
NT = 512         # query tile (free dim of S^T / output matmuls)
NNT = NQ // NT   # 4 query tiles
NSC = 8          # key super-chunks of 512 (4 x 128) keys
F32 = mybir.dt.float32
F32R = mybir.dt.float32r
AFT = mybir.ActivationFunctionType


def attn_tile_kernel(tc, out, x1, x2, wq4t, wk4t, wvt, bq4, bk4, bv, ones_c):
    nc = tc.nc

    with (
        tc.tile_pool(name="consts", bufs=1) as consts,
        tc.tile_pool(name="bigbuf", bufs=1) as bigbuf,
        tc.tile_pool(name="ptbuf", bufs=2) as ptbuf,
        tc.tile_pool(name="finbuf", bufs=2) as finbuf,
    ):
        # ---- constants / weights -------------------------------------
        ones_rs = consts.tile([128, 1], F32R, name="ones_rs")
        nc.sync.dma_start(out=ones_rs, in_=ones_c[:, 0:1])
        ones_bc = consts.tile([1, 128], F32R, name="ones_bc")
        nc.sync.dma_start(out=ones_bc, in_=ones_c[0:1, :])

        bq4_sb = consts.tile([128, 1], F32, name="bq4_sb")
        nc.sync.dma_start(out=bq4_sb, in_=bq4)
        bk4_sb = consts.tile([128, 1], F32, name="bk4_sb")
        nc.sync.dma_start(out=bk4_sb, in_=bk4)
        bv_sb = []
        for cc in range(2):
            t = consts.tile([128, 1], F32, name=f"bv_sb{cc}")
            nc.sync.dma_start(out=t, in_=bv[cc * 128 : (cc + 1) * 128, :])
            bv_sb.append(t)

        wq4t_sb, wk4t_sb, wvt_sb = [], [], []
        for kc in range(2):
            rows = slice(kc * 128, (kc + 1) * 128)
            t = consts.tile([128, 128], F32R, name=f"wq4t_sb{kc}")
            nc.sync.dma_start(out=t, in_=wq4t[rows, :])
            wq4t_sb.append(t)
            t = consts.tile([128, 128], F32R, name=f"wk4t_sb{kc}")
            nc.sync.dma_start(out=t, in_=wk4t[rows, :])
            wk4t_sb.append(t)
            t = consts.tile([128, 256], F32R, name=f"wvt_sb{kc}")
            nc.sync.dma_start(out=t, in_=wvt[rows, :])
            wvt_sb.append(t)

        # ---- feature maps --------------------------------------------
        x1_sb, x2_sb = [], []
        for kc in range(2):
            rows = slice(kc * 128, (kc + 1) * 128)
            t = bigbuf.tile([128, NQ], F32R, name=f"x1_sb{kc}")
            for blk in range(2):
                cols = slice(blk * 1024, (blk + 1) * 1024)
                nc.sync.dma_start(out=t[:, cols], in_=x1[rows, cols])
            x1_sb.append(t)
            t = bigbuf.tile([128, N], F32R, name=f"x2_sb{kc}")
            for blk in range(4):
                cols = slice(blk * 1024, (blk + 1) * 1024)
                nc.sync.dma_start(out=t[:, cols], in_=x2[rows, cols])
            x2_sb.append(t)

        q4_sb = bigbuf.tile([128, NQ], F32R, name="q4_sb")
        k4_sb = bigbuf.tile([128, N], F32R, name="k4_sb")
        vt_sb = bigbuf.tile([128, C * N // 128], F32R, name="vt_sb")  # [128, 8192]

        # ---- prep: projections ---------------------------------------
        with tc.tile_pool(name="prep_psum", bufs=2, space="PSUM") as pp:
            # Q4 [128, 2048] = (Wq stacked 4x) @ x1, then +bq
            psum_q = pp.tile([128, NQ], F32, name="psum_q", tag="prep")
            for t4 in range(NNT):
                cols = slice(t4 * NT, (t4 + 1) * NT)
                for kc in range(2):
                    nc.tensor.matmul(
                        psum_q[:, cols],
                        lhsT=(wq4t_sb[kc]),
                        rhs=(x1_sb[kc][:, cols]),
                        start=(kc == 0),
                        stop=(kc == 1),
                    )
            nc.vector.tensor_scalar_add(q4_sb, psum_q, bq4_sb)

            # K4 [128, 4096] in two passes
            for p in range(2):
                psum_k = pp.tile([128, NQ], F32, name=f"psum_k{p}", tag="prep")
                for t4 in range(NNT):
                    cols = slice(t4 * NT, (t4 + 1) * NT)
                    src = slice(p * NQ + t4 * NT, p * NQ + (t4 + 1) * NT)
                    for kc in range(2):
                        nc.tensor.matmul(
                            psum_k[:, cols],
                            lhsT=(wk4t_sb[kc]),
                            rhs=(x2_sb[kc][:, src]),
                            start=(kc == 0),
                            stop=(kc == 1),
                        )
                nc.vector.tensor_scalar_add(
                    k4_sb[:, p * NQ : (p + 1) * NQ], psum_k, bk4_sb
                )

            # V^T: vt_sb[:, mc*256 + c] = V[c, mc*128 + p] ; 4 passes of 8
            # m-chunks.  (bv is added at the very end of the kernel.)
            for p in range(4):
                psum_v = pp.tile([128, NQ], F32, name=f"psum_v{p}", tag="prep")
                for m8 in range(8):
                    mc = 8 * p + m8
                    for kc in range(2):
                        nc.tensor.matmul(
                            psum_v[:, m8 * 256 : (m8 + 1) * 256],
                            lhsT=(x2_sb[kc][:, mc * 128 : (mc + 1) * 128]),
                            rhs=(wvt_sb[kc]),
                            start=(kc == 0),
                            stop=(kc == 1),
                        )
                dst = vt_sb[:, p * 2048 : (p + 1) * 2048]
                if p % 2 == 0:
                    nc.scalar.copy(dst, psum_v)
                else:
                    nc.vector.tensor_copy(dst, psum_v)

        # ---- main attention loop -------------------------------------
        with (
            tc.tile_pool(name="s_psum", bufs=1, space="PSUM") as sp,
            tc.tile_pool(name="o_psum", bufs=1, space="PSUM") as op,
            tc.tile_pool(name="b_psum", bufs=1, space="PSUM") as bp,
        ):
            for nt in range(NNT):
                qcols = slice(nt * NT, (nt + 1) * NT)
                psum_o0 = op.tile([128, NT], F32, name=f"po0_{nt}", tag="o0")
                psum_o1 = op.tile([128, NT], F32, name=f"po1_{nt}", tag="o1")
                psum_rs = op.tile([1, NT], F32, name=f"prs_{nt}", tag="rs")
                psum_o = [psum_o0, psum_o1]

                def _emit_av(sc, pt):
                    first, last = sc == 0, sc == NSC - 1
                    for j in range(4):
                        pcols = slice(j * NT, (j + 1) * NT)
                        nc.tensor.matmul(
                            psum_rs,
                            lhsT=(ones_rs),
                            rhs=(pt[:, pcols]),
                            start=(first and j == 0),
                            stop=(last and j == 3),
                        )
                    for j in range(4):
                        mc = 4 * sc + j
                        pcols = slice(j * NT, (j + 1) * NT)
                        for cc in range(2):
                            nc.tensor.matmul(
                                psum_o[cc],
                                lhsT=(
                                    vt_sb[:, mc * 256 + cc * 128 : mc * 256 + (cc + 1) * 128]
                                ),
                                rhs=(pt[:, pcols]),
                                start=(first and j == 0),
                                stop=(last and j == 3),
                            )

                prev = None
                for sc in range(NSC):
                    # S^T super-chunk: 4 row-packed matmuls, contract dim 32
                    psum_s = sp.tile([128, 4 * NT], F32, name=f"ps_{nt}_{sc}", tag="s")
                    for j in range(4):
                        mc = 4 * sc + j
                        rowg = slice(32 * j, 32 * (j + 1))
                        nc.tensor.matmul(
                            psum_s[:, j * NT : (j + 1) * NT],
                            lhsT=(k4_sb[rowg, mc * 128 : (mc + 1) * 128]),
                            rhs=(q4_sb[rowg, qcols]),
                            start=True,
                            stop=True,
                            tile_position=(32 * j, 0),
                        )
                    pt = ptbuf.tile([128, 4 * NT], F32R, name=f"pt_{nt}_{sc}", tag="pt")
                    nc.scalar.activation(out=pt, in_=psum_s, func=AFT.Exp)
                    if prev is not None:
                        _emit_av(*prev)
                    prev = (sc, pt)
                _emit_av(*prev)

                # ---- finalize: normalize + bias + store ----------------
                # evacuate PSUM fast (frees banks for the next n-tile), then
                # normalize on SBUF off the PE critical path
                rs_sb = finbuf.tile([1, NT], F32R, name=f"rs_sb_{nt}", tag="rs_sb")
                nc.scalar.copy(rs_sb, psum_rs)
                raw0 = finbuf.tile([128, NT], F32, name=f"raw0_{nt}", tag="raw0")
                nc.scalar.copy(raw0, psum_o0)
                raw1 = finbuf.tile([128, NT], F32, name=f"raw1_{nt}", tag="raw1")
                nc.vector.tensor_copy(raw1, psum_o1)
                raw = [raw0, raw1]
                psum_b = bp.tile([128, NT], F32, name=f"pb_{nt}", tag="b")
                nc.tensor.matmul(
                    psum_b, lhsT=ones_bc, rhs=rs_sb, start=True, stop=True
                )
                rbc = finbuf.tile([128, NT], F32, name=f"rbc_{nt}", tag="rbc")
                nc.vector.reciprocal_approx_fast(out=rbc, in_=psum_b)
                for cc in range(2):
                    t_sb = finbuf.tile([128, NT], F32, name=f"t_{nt}_{cc}", tag=f"t{cc}")
                    nc.vector.tensor_mul(t_sb, raw[cc], rbc)
                    o_sb = finbuf.tile([128, NT], F32, name=f"o_{nt}_{cc}", tag=f"o{cc}")
                    nc.scalar.add(o_sb, t_sb, bv_sb[cc])
                    nc.sync.dma_start(
                        out=out[cc * 128 : (cc + 1) * 128, qcols], in_=o_sb
                    )


def build_nc():
    nc = bacc.Bacc("TRN2", target_bir_lowering=False, debug=False)
    x1 = nc.dram_tensor("x1", [C, NQ], F32R, kind="ExternalInput").ap()
    x2 = nc.dram_tensor("x2", [C, N], F32R, kind="ExternalInput").ap()
    wq4t = nc.dram_tensor("wq4t", [C, 128], F32R, kind="ExternalInput").ap()
    wk4t = nc.dram_tensor("wk4t", [C, 128], F32R, kind="ExternalInput").ap()
    wvt = nc.dram_tensor("wvt", [C, C], F32R, kind="ExternalInput").ap()
    bq4 = nc.dram_tensor("bq4", [128, 1], F32, kind="ExternalInput").ap()
    bk4 = nc.dram_tensor("bk4", [128, 1], F32, kind="ExternalInput").ap()
    bv = nc.dram_tensor("bv", [C, 1], F32, kind="ExternalInput").ap()
    ones_c = nc.dram_tensor("ones_c", [128, 128], F32R, kind="ExternalInput").ap()
    out = nc.dram_tensor("out", [C, NQ], F32, kind="ExternalOutput").ap()
    with tile.TileContext(nc) as tc:
        attn_tile_kernel(tc, out, x1, x2, wq4t, wk4t, wvt, bq4, bk4, bv, ones_c)
    nc.compile()
    return nc


def make_in_maps(f1, f2, Wq, bq, Wk, bk, Wv, bv):
    f1 = np.asarray(f1, dtype=np.float32)
    f2 = np.asarray(f2, dtype=np.float32)
    Wq = np.asarray(Wq, dtype=np.float32)
    Wk = np.asarray(Wk, dtype=np.float32)
    Wv = np.asarray(Wv, dtype=np.float32)
    bq = np.asarray(bq, dtype=np.float32)
    bk = np.asarray(bk, dtype=np.float32)
    bv = np.asarray(bv, dtype=np.float32)

    x1 = f1.reshape(B, C, N)
    x2 = f2.reshape(B, C, N)
    wq4t = np.ascontiguousarray(np.concatenate([Wq.T] * 4, axis=1))  # [256, 128]
    wk4t = np.ascontiguousarray(np.concatenate([Wk.T] * 4, axis=1))
    wvt = np.ascontiguousarray(Wv.T)                                 # [256, 256]
    bq4 = np.ascontiguousarray(np.tile(bq, 4).reshape(128, 1))
    bk4 = np.ascontiguousarray(np.tile(bk, 4).reshape(128, 1))
    bvv = np.ascontiguousarray(bv.reshape(C, 1))
    ones_c = np.ones((128, 128), np.float32)

    in_maps = []
    for core in range(8):
        b, h = divmod(core, 2)
        in_maps.append(
            dict(
                x1=np.ascontiguousarray(x1[b, :, h * NQ : (h + 1) * NQ]),
                x2=np.ascontiguousarray(x2[b]),
                wq4t=wq4t,
                wk4t=wk4t,
                wvt=wvt,
                bq4=bq4,
                bk4=bk4,
                bv=bvv,
                ones_c=ones_c,
            )
        )
    return in_maps


_NC_CACHE = None


def _get_nc():
    global _NC_CACHE
    if _NC_CACHE is None:
        _NC_CACHE = build_nc()
    return _NC_CACHE


def kernel(f1, f2, Wq, bq, Wk, bk, Wv, bv):
    in_maps = make_in_maps(f1, f2, Wq, bq, Wk, bk, Wv, bv)
    res = bass_utils.run_bass_kernel_spmd(_get_nc(), in_maps, core_ids=list(range(8)))
    out = np.empty((B, C, N), np.float32)
    for core in range(8):
        b, h = divmod(core, 2)
        out[b, :, h * NQ : (h + 1) * NQ] = res.results[core]["out"]
    return out.reshape(B, C, 64, 64)


# revision 13
# speedup vs baseline: 1.2385x; 1.1937x over previous
"""Cross-attention Trainium2 kernel (nn_CrossAttention).

Reference computation (per batch b):
    q = Wq @ x1 + bq            [32, N]     (N = 64*64 = 4096)
    k = Wk @ x2 + bk            [32, N]
    v = Wv @ x2 + bv            [256, N]
    attn = softmax(q^T k, axis over keys m)     [N, N]
    out[c, n] = sum_m v[c, m] attn[n, m]        [256, N]

Sharding: 8 cores = 4 batches x 2 query-halves (2048 queries per core, all
4096 keys).  Each core runs the same NEFF on its own input slice; softmax
rows are complete within a core so no cross-core communication is needed.

Per-core kernel layout choices:
  * S^T tiles [keys m on partitions, queries n on free dim] so that the
    second matmul (attn @ V) can consume exp(S^T) directly from SBUF with m
    as the contraction dim -- no transposes anywhere.
  * Q and K are produced replicated 4x across partition groups (Wq/Wk
    stacked 4x on the host) so the D=32-contraction QK^T matmuls can be
    row-packed 4-per-PE-array via tile_position.  Projections and QK^T run
    in float32r (full fp32 data, reduced-precision PE mode, full rate for
    free dim >= 256) to keep logit precision high.
  * exp(S^T) is written in bf16: attention weights tolerate the 0.4%
    rounding, AV matmuls stream bf16 at full rate with fast weight loads
    (V^T is bf16 too), and softmax row sums use col-packed M=1
    ones-matmuls (4 concurrent via tile_position) in one PSUM bank.
    PSUM accumulation stays fp32.
  * Row-sum partials land on partitions {0,32,64,96}; a DMA gather + K=4
    ones-matmul combines and broadcasts them to all partitions, then a
    fast approximate reciprocal (~18 significant bits) normalizes.
  * Main loop is a flat software pipeline over (n-tile, key-super-chunk)
    steps: AV matmuls of step i are emitted after step i+1's S^T + exp, so
    the PE stays busy across n-tile seams (keeps the HAM clock-gate warm).
  * Softmax skips the max-subtraction: logits are ~N(0, 32), |s| < ~45
    for this problem size, exp() stays comfortably inside fp32/bf16 range.
  * bv is folded in at the end: out += bv (softmax rows sum to 1).
"""

import sys

for _p in (
    "/root/.axon_site",
    "/root/.axon_site/_ro/trn_rl_repo",
    "/root/.axon_site/_ro/pypackages",
):
    if _p not in sys.path:
        sys.path.append(_p)

import numpy as np

import concourse.bass as bass
from concourse import bacc
import concourse.tile as tile
from concourse import mybir
from concourse import bass_utils

B = 4
C = 256          # value/input channels
D = 32           # q/k channels
N = 4096         # keys per batch (64*64)
NQ = 2048        # queries per core (half a batch)
NT = 512         # query tile (free dim of S^T / output matmuls)
NNT = NQ // NT   # 4 query tiles
NSC = 8          # key super-chunks of 512 (4 x 128) keys
F32 = mybir.dt.float32
F32R = mybir.dt.float32r
BF16 = mybir.dt.bfloat16
AFT = mybir.ActivationFunctionType


def attn_tile_kernel(tc, out, x1, x2, wq4t, wk4t, wvt, bq4, bk4, bv, ones_c, ones_f):
    nc = tc.nc

    with (
        tc.tile_pool(name="consts", bufs=1) as consts,
        tc.tile_pool(name="bigbuf", bufs=1) as bigbuf,
        tc.tile_pool(name="ptbuf", bufs=2) as ptbuf,
        tc.tile_pool(name="finbuf", bufs=2) as finbuf,
    ):
        # ---- constants / weights -------------------------------------
        ones_rs = consts.tile([128, 32], BF16, name="ones_rs")
        nc.sync.dma_start(out=ones_rs, in_=ones_c)
        ones_bc = consts.tile([4, 128], F32R, name="ones_bc")
        nc.sync.dma_start(out=ones_bc, in_=ones_f)

        bq4_sb = consts.tile([128, 1], F32, name="bq4_sb")
        nc.sync.dma_start(out=bq4_sb, in_=bq4)
        bk4_sb = consts.tile([128, 1], F32, name="bk4_sb")
        nc.sync.dma_start(out=bk4_sb, in_=bk4)
        bv_sb = []
        for cc in range(2):
            t = consts.tile([128, 1], F32, name=f"bv_sb{cc}")
            nc.sync.dma_start(out=t, in_=bv[cc * 128 : (cc + 1) * 128, :])
            bv_sb.append(t)

        wq4t_sb, wk4t_sb, wvt_sb = [], [], []
        for kc in range(2):
            rows = slice(kc * 128, (kc + 1) * 128)
            t = consts.tile([128, 128], F32R, name=f"wq4t_sb{kc}")
            nc.sync.dma_start(out=t, in_=wq4t[rows, :])
            wq4t_sb.append(t)
            t = consts.tile([128, 128], F32R, name=f"wk4t_sb{kc}")
            nc.scalar.dma_start(out=t, in_=wk4t[rows, :])
            wk4t_sb.append(t)
            t = consts.tile([128, 256], F32R, name=f"wvt_sb{kc}")
            nc.scalar.dma_start(out=t, in_=wvt[rows, :])
            wvt_sb.append(t)

        # ---- feature maps (x1 first: Q4 is on the critical path) -----
        x1_sb = [
            bigbuf.tile([128, NQ], F32R, name="x1_sb0"),
            bigbuf.tile([128, NQ], F32R, name="x1_sb1"),
        ]
        x2_sb = [
            bigbuf.tile([128, N], F32R, name="x2_sb0"),
            bigbuf.tile([128, N], F32R, name="x2_sb1"),
        ]
        for blk in range(2):
            cols = slice(blk * 1024, (blk + 1) * 1024)
            nc.sync.dma_start(out=x1_sb[0][:, cols], in_=x1[0:128, cols])
            nc.scalar.dma_start(out=x1_sb[1][:, cols], in_=x1[128:256, cols])
        for blk in range(4):
            cols = slice(blk * 1024, (blk + 1) * 1024)
            nc.sync.dma_start(out=x2_sb[0][:, cols], in_=x2[0:128, cols])
            nc.scalar.dma_start(out=x2_sb[1][:, cols], in_=x2[128:256, cols])

        q4_sb = bigbuf.tile([128, NQ], F32R, name="q4_sb")
        k4_sb = bigbuf.tile([128, N], F32R, name="k4_sb")
        vt_sb = bigbuf.tile([128, C * N // 128], BF16, name="vt_sb")  # [128, 8192]

        # ---- prep: projections ---------------------------------------
        with tc.tile_pool(name="prep_psum", bufs=2, space="PSUM") as pp:
            # Q4 [128, 2048] = (Wq stacked 4x) @ x1, then +bq
            psum_q = pp.tile([128, NQ], F32, name="psum_q", tag="prep")
            for t4 in range(NNT):
                cols = slice(t4 * NT, (t4 + 1) * NT)
                for kc in range(2):
                    nc.tensor.matmul(
                        psum_q[:, cols],
                        lhsT=wq4t_sb[kc],
                        rhs=x1_sb[kc][:, cols],
                        start=(kc == 0),
                        stop=(kc == 1),
                    )
            nc.vector.tensor_scalar_add(q4_sb, psum_q, bq4_sb)

            # K4 [128, 4096] in two passes
            for p in range(2):
                psum_k = pp.tile([128, NQ], F32, name=f"psum_k{p}", tag="prep")
                for t4 in range(NNT):
                    cols = slice(t4 * NT, (t4 + 1) * NT)
                    src = slice(p * NQ + t4 * NT, p * NQ + (t4 + 1) * NT)
                    for kc in range(2):
                        nc.tensor.matmul(
                            psum_k[:, cols],
                            lhsT=wk4t_sb[kc],
                            rhs=x2_sb[kc][:, src],
                            start=(kc == 0),
                            stop=(kc == 1),
                        )
                nc.vector.tensor_scalar_add(
                    k4_sb[:, p * NQ : (p + 1) * NQ], psum_k, bk4_sb
                )

            # V^T (bf16): vt_sb[:, mc*256 + c] = V[c, mc*128 + p]; 4 passes
            # of 8 m-chunks, copied out in halves alternating ACT/DVE so AV
            # matmuls unblock at 1024-column granularity.
            for p in range(4):
                psum_v = pp.tile([128, NQ], F32, name=f"psum_v{p}", tag="prep")
                for m8 in range(8):
                    mc = 8 * p + m8
                    for kc in range(2):
                        nc.tensor.matmul(
                            psum_v[:, m8 * 256 : (m8 + 1) * 256],
                            lhsT=x2_sb[kc][:, mc * 128 : (mc + 1) * 128],
                            rhs=wvt_sb[kc],
                            start=(kc == 0),
                            stop=(kc == 1),
                        )
                for h in range(2):
                    cols = slice(h * 1024, (h + 1) * 1024)
                    dst = vt_sb[:, p * 2048 + h * 1024 : p * 2048 + (h + 1) * 1024]
                    if h == 0:
                        nc.scalar.copy(dst, psum_v[:, cols])
                    else:
                        nc.vector.tensor_copy(dst, psum_v[:, cols])

        # ---- main attention loop -------------------------------------
        # Flat software pipeline over (nt, sc) steps: the AV/rowsum matmuls
        # for step i are emitted after step i+1's S^T matmuls + exp, so the
        # PE always has work while ACT computes exp -- including across
        # n-tile boundaries (keeps the HAM clock-gate warm).
        with (
            tc.tile_pool(name="s_psum", bufs=1, space="PSUM") as sp,
            tc.tile_pool(name="o_psum", bufs=1, space="PSUM") as op,
            tc.tile_pool(name="b_psum", bufs=1, space="PSUM") as bp,
        ):
            state = {}

            def _emit_st(nt, sc):
                # S^T super-chunk: 4 row-packed matmuls, contract dim 32
                qcols = slice(nt * NT, (nt + 1) * NT)
                psum_s = sp.tile([128, 4 * NT], F32, name=f"ps_{nt}_{sc}", tag="s")
                for j in range(4):
                    mc = 4 * sc + j
                    rowg = slice(32 * j, 32 * (j + 1))
                    nc.tensor.matmul(
                        psum_s[:, j * NT : (j + 1) * NT],
                        lhsT=k4_sb[rowg, mc * 128 : (mc + 1) * 128],
                        rhs=q4_sb[rowg, qcols],
                        start=True,
                        stop=True,
                        tile_position=(32 * j, 0),
                    )
                pt = ptbuf.tile([128, 4 * NT], BF16, name=f"pt_{nt}_{sc}", tag="pt")
                nc.scalar.activation(out=pt, in_=psum_s, func=AFT.Exp)
                return pt

            def _emit_av(nt, sc, pt):
                first, last = sc == 0, sc == NSC - 1
                if first:
                    state[nt] = (
                        op.tile([128, NT], F32, name=f"po0_{nt}", tag="o0"),
                        op.tile([128, NT], F32, name=f"po1_{nt}", tag="o1"),
                        op.tile([128, NT], F32, name=f"prs_{nt}", tag="rs"),
                    )
                psum_o0, psum_o1, psum_rs = state[nt]
                for j in range(4):
                    # col-packed rowsums: 4 concurrent M=1 tiles, partials
                    # land on partitions {0, 32, 64, 96}
                    pcols = slice(j * NT, (j + 1) * NT)
                    nc.tensor.matmul(
                        psum_rs[32 * j : 32 * (j + 1), :],
                        lhsT=ones_rs,
                        rhs=pt[:, pcols],
                        start=first,
                        stop=last,
                        tile_position=(0, 32 * j),
                        skip_group_check=True,
                    )
                for j in range(4):
                    mc = 4 * sc + j
                    pcols = slice(j * NT, (j + 1) * NT)
                    for cc in range(2):
                        nc.tensor.matmul(
                            (psum_o0, psum_o1)[cc],
                            lhsT=vt_sb[
                                :, mc * 256 + cc * 128 : mc * 256 + (cc + 1) * 128
                            ],
                            rhs=pt[:, pcols],
                            start=(first and j == 0),
                            stop=(last and j == 3),
                        )

            def _emit_fin(nt):
                # evacuate PSUM fast (frees banks for the next tile), then
                # normalize on SBUF off the PE critical path
                psum_o0, psum_o1, psum_rs = state.pop(nt)
                qcols = slice(nt * NT, (nt + 1) * NT)
                rs_sb = finbuf.tile([128, NT], F32R, name=f"rs_sb_{nt}", tag="rs_sb")
                nc.scalar.copy(rs_sb, psum_rs)
                raw0 = finbuf.tile([128, NT], F32, name=f"raw0_{nt}", tag="raw0")
                nc.scalar.copy(raw0, psum_o0)
                raw1 = finbuf.tile([128, NT], F32, name=f"raw1_{nt}", tag="raw1")
                nc.vector.tensor_copy(raw1, psum_o1)
                # gather the 4 partial rows onto adjacent partitions, then a
                # K=4 ones-matmul combines + broadcasts to all 128 partitions
                rs4p = finbuf.tile([4, NT], F32R, name=f"rs4p_{nt}", tag="rs4p")
                nc.sync.dma_start(out=rs4p, in_=rs_sb[0:97:32, :])
                psum_b = bp.tile([128, NT], F32, name=f"pb_{nt}", tag="b")
                nc.tensor.matmul(
                    psum_b, lhsT=ones_bc, rhs=rs4p, start=True, stop=True
                )
                rbc = finbuf.tile([128, NT], F32, name=f"rbc_{nt}", tag="rbc")
                nc.vector.reciprocal_approx_fast(out=rbc, in_=psum_b)
                for cc, raw in ((0, raw0), (1, raw1)):
                    t_sb = finbuf.tile([128, NT], F32, name=f"t_{nt}_{cc}", tag=f"t{cc}")
                    nc.vector.tensor_mul(t_sb, raw, rbc)
                    o_sb = finbuf.tile([128, NT], F32, name=f"o_{nt}_{cc}", tag=f"o{cc}")
                    nc.scalar.add(o_sb, t_sb, bv_sb[cc])
                    nc.sync.dma_start(
                        out=out[cc * 128 : (cc + 1) * 128, qcols], in_=o_sb
                    )

            steps = [(nt, sc) for nt in range(NNT) for sc in range(NSC)]
            prev = None
            for nt, sc in steps:
                pt = _emit_st(nt, sc)
                if prev is not None:
                    _emit_av(*prev)
                    if prev[1] == NSC - 1:
                        _emit_fin(prev[0])
                prev = (nt, sc, pt)
            _emit_av(*prev)
            _emit_fin(prev[0])


def build_nc():
    nc = bacc.Bacc("TRN2", target_bir_lowering=False, debug=False)
    x1 = nc.dram_tensor("x1", [C, NQ], F32R, kind="ExternalInput").ap()
    x2 = nc.dram_tensor("x2", [C, N], F32R, kind="ExternalInput").ap()
    wq4t = nc.dram_tensor("wq4t", [C, 128], F32R, kind="ExternalInput").ap()
    wk4t = nc.dram_tensor("wk4t", [C, 128], F32R, kind="ExternalInput").ap()
    wvt = nc.dram_tensor("wvt", [C, C], F32R, kind="ExternalInput").ap()
    bq4 = nc.dram_tensor("bq4", [128, 1], F32, kind="ExternalInput").ap()
    bk4 = nc.dram_tensor("bk4", [128, 1], F32, kind="ExternalInput").ap()
    bv = nc.dram_tensor("bv", [C, 1], F32, kind="ExternalInput").ap()
    ones_cd = nc.dram_tensor("ones_c", [128, 32], BF16, kind="ExternalInput").ap()
    ones_fd = nc.dram_tensor("ones_f", [4, 128], F32R, kind="ExternalInput").ap()
    out = nc.dram_tensor("out", [C, NQ], F32, kind="ExternalOutput").ap()
    with tile.TileContext(nc) as tc:
        attn_tile_kernel(
            tc, out, x1, x2, wq4t, wk4t, wvt, bq4, bk4, bv, ones_cd, ones_fd
        )
    nc.compile()
    return nc


def make_in_maps(f1, f2, Wq, bq, Wk, bk, Wv, bv):
    f1 = np.asarray(f1, dtype=np.float32)
    f2 = np.asarray(f2, dtype=np.float32)
    Wq = np.asarray(Wq, dtype=np.float32)
    Wk = np.asarray(Wk, dtype=np.float32)
    Wv = np.asarray(Wv, dtype=np.float32)
    bq = np.asarray(bq, dtype=np.float32)
    bk = np.asarray(bk, dtype=np.float32)
    bv = np.asarray(bv, dtype=np.float32)

    x1 = f1.reshape(B, C, N)
    x2 = f2.reshape(B, C, N)
    wq4t = np.ascontiguousarray(np.concatenate([Wq.T] * 4, axis=1))  # [256, 128]
    wk4t = np.ascontiguousarray(np.concatenate([Wk.T] * 4, axis=1))
    wvt = np.ascontiguousarray(Wv.T)                                 # [256, 256]
    bq4 = np.ascontiguousarray(np.tile(bq, 4).reshape(128, 1))
    bk4 = np.ascontiguousarray(np.tile(bk, 4).reshape(128, 1))
    bvv = np.ascontiguousarray(bv.reshape(C, 1))
    import ml_dtypes

    ones_c = np.ones((128, 32), ml_dtypes.bfloat16)
    ones_f = np.ones((4, 128), np.float32)

    in_maps = []
    for core in range(8):
        b, h = divmod(core, 2)
        in_maps.append(
            dict(
                x1=np.ascontiguousarray(x1[b, :, h * NQ : (h + 1) * NQ]),
                x2=np.ascontiguousarray(x2[b]),
                wq4t=wq4t,
                wk4t=wk4t,
                wvt=wvt,
                bq4=bq4,
                bk4=bk4,
                bv=bvv,
                ones_c=ones_c,
                ones_f=ones_f,
            )
        )
    return in_maps


_NC_CACHE = None


def _get_nc():
    global _NC_CACHE
    if _NC_CACHE is None:
        _NC_CACHE = build_nc()
    return _NC_CACHE


def kernel(f1, f2, Wq, bq, Wk, bk, Wv, bv):
    in_maps = make_in_maps(f1, f2, Wq, bq, Wk, bk, Wv, bv)
    res = bass_utils.run_bass_kernel_spmd(_get_nc(), in_maps, core_ids=list(range(8)))
    out = np.empty((B, C, N), np.float32)
    for core in range(8):
        b, h = divmod(core, 2)
        out[b, :, h * NQ : (h + 1) * NQ] = res.results[core]["out"]
    return out.reshape(B, C, 64, 64)
